# revision 4
# baseline (speedup 1.0000x reference)
"""GGML Q8_0 fused dequant + mat-vec kernel for Trainium2 (8 NeuronCores).

out[b, o] = sum_{k} x[b, k] * scales[o, k//32] * q[o, k] + bias[o]
  x: [1, 4096] f32, q: [14336, 4096] int32 (int8 values), scales: [14336, 128] f32,
  bias: [14336] f32 -> out [1, 14336] f32

Sharding: row-parallel (out_features) across 8 cores; x replicated.

Per-core program (TensorE-centric, int8 transport; ~23 us/pass on HW,
essentially the HBM roofline for the 7.3 MB/core of int8 weights):
  host ships qT [4096, 1792] int8 (k-major transpose of the core's rows) --
  4x less HBM traffic than the int32 the harness hands us.
  k-tiles (128 k's) are DMA'd in groups of 4 and upcast int8 -> fp16 in one
  instruction per group, alternating DVE / ACT so both engines convert in
  parallel with the PE.
  Stage 1: per k-tile matmul with stationary x_diag [128, 128] (fp16 x
    values masked per 32-block) -> per-block partials P_all[4kt+m, row]
    accumulated in PSUM.  The PE does the multiply AND the intra-block
    32-element reduction in one streaming pass, replacing the DVE
    mul+add-tree of the row-parallel formulation.
  Stage 2: DVE: M = P_all * scalesT (per-block scales); a ones-matmul
    reduces over the 128 block-partitions; DVE adds bias; DMA out.
  x_diag is built on device by memset + 4 strided lattice DMAs (only the
  4096 nonzeros cross the DMA queue, not a 1 MB zero-filled image).
"""

import sys

import numpy as np

if "/opt/trn_rl_repo" not in sys.path:
    sys.path.insert(0, "/opt/trn_rl_repo")

OUT_F = 14336
IN_F = 4096
BLOCK = 32
NB = IN_F // BLOCK  # 128 blocks per row
N_CORES = 8
ROWS = OUT_F // N_CORES  # 1792 rows per core
P = 128  # partitions
KT = IN_F // P  # 32 k-tiles
G = 4  # k-tiles per DMA/convert group
NG = KT // G  # 8 groups
CH = 448  # row-chunk size (fits one PSUM bank as f32)
NCH = ROWS // CH  # 4 chunks
# groups converted on DVE (rest on ACT); DVE is faster, interleave for overlap
DVE_GROUPS = (0, 1, 2, 4, 6)

_NC_CACHE = {}


def _patch_tile_exit_drain():
    """Split the TileContext exit-drain sem waits across 1-wait NOPs.

    The walrus in this container lowers SP CTRL (NoOp/Drain) instructions
    with at most ONE sync-wait command; Tile's kernel-tail drain attaches a
    wait per live semaphore to a single instruction, which fails codegen
    with "Too many sync wait commands".  Redistribute the waits across a
    chain of SP NOPs (sequential on the SP stream, so ordering semantics
    are preserved) before the drain.
    """
    import concourse.mybir as mybir
    import concourse.tile as tile

    if getattr(tile.TileContext, "_ant_drain_patch", False):
        return

    def _drain_and_barrier(self, tick_clock, wait_clock):
        nc = self.nc
        carrier = nc.sync.nop(nofuse=True)
        wait_clock.add_sem_waits(
            carrier.ins, tile.ScopedClock({None: tick_clock.global_clock}))
        si = carrier.ins.sync_info
        waits = list(si.on_wait) if si is not None else []
        if len(waits) > 1:
            carrier.ins.sync_info = mybir.SyncInfo(
                on_wait=waits[:1], on_update=list(si.on_update))
            for i in range(1, len(waits)):
                extra = nc.sync.nop(nofuse=True)
                extra.ins.sync_info = mybir.SyncInfo(
                    on_wait=waits[i:i + 1], on_update=[])
        nc.sync.drain()
        nc.all_engine_barrier()
        assert self.sems is not None
        popped = nc._tile_sem_poison_stack.pop()
        assert popped is self._sem_poison
        nc.clear_and_free_semaphores(list(self.sems.allocated().values()))
        nc.all_engine_barrier()

    tile.TileContext._drain_and_barrier = _drain_and_barrier
    tile.TileContext._ant_drain_patch = True


def _legalize_sync_waits(nc):
    """Split multi-wait instructions for a walrus that encodes one sync wait.

    Tile's semaphore assignment may attach several sem waits to one
    instruction; this walrus build rejects >1 ("Too many sync wait
    commands").  Hoist all but the last wait onto NoOp instructions injected
    just before the instruction on the same engine (engine streams execute
    in order, so the wait semantics are unchanged).
    """
    import concourse.mybir as mybir

    n_split = 0
    for f in nc.m.functions:
        for bb in f.blocks:
            il = bb.instructions
            if not any(
                ins.sync_info is not None and len(ins.sync_info.on_wait) > 1
                for ins in il
            ):
                continue
            new = []
            for ins in il:
                si = ins.sync_info
                if si is not None and len(si.on_wait) > 1:
                    waits = list(si.on_wait)
                    for w in waits[:-1]:
                        nop = mybir.InstNoOp(
                            name=f"I-waitnop-{nc.next_id()}", ins=[], outs=[])
                        nop.engine = ins.engine
                        nop.sync_info = mybir.SyncInfo(
                            on_wait=[w], on_update=[])
                        nc.register_instruction(nop, overwrite=True)
                        new.append(nop)
                        n_split += 1
                    ins.sync_info = mybir.SyncInfo(
                        on_wait=[waits[-1]], on_update=list(si.on_update))
                new.append(ins)
            il[:] = new
    return n_split


def _build_nc(passes=1):
    """Build the per-core Bass program.

    passes>1 repeats the whole computation inside one NEFF, accumulating the
    per-pass result into the output (out = passes * y), used only by the
    benchmark harness to measure steady-state per-pass device time.
    """
    if passes in _NC_CACHE:
        return _NC_CACHE[passes]

    import concourse.bass as bass
    import concourse.mybir as mybir
    import concourse.tile as tile

    _patch_tile_exit_drain()

    f32 = mybir.dt.float32
    fp16 = mybir.dt.float16
    i8 = mybir.dt.int8
    Copy = mybir.ActivationFunctionType.Copy

    nc = bass.Bass("TRN2", target_bir_lowering=False, debug=False,
                   num_devices=N_CORES)

    # qT stored group-major [NG, P, G, ROWS]: each partition's group data is
    # one contiguous 7168 B run -> 128 fat descriptors per group DMA instead
    # of 512 thin ones (less packet + metadata overhead per byte)
    qt_d = nc.dram_tensor("qt", [NG, P, G, ROWS], i8,
                          kind="ExternalInput").ap()
    xds_d = nc.dram_tensor("xds", [P, KT * P], fp16,
                           kind="ExternalInput").ap()
    sct_d = nc.dram_tensor("sct", [P, ROWS], fp16, kind="ExternalInput").ap()
    ones_d = nc.dram_tensor("onesb", [P, NCH * 4], fp16,
                            kind="ExternalInput").ap()
    bias_d = nc.dram_tensor("bias4", [NCH, CH], f32, kind="ExternalInput").ap()
    out_d = nc.dram_tensor("out", [NCH, CH], f32, kind="ExternalOutput").ap()

    with tile.TileContext(nc) as tc:
        with (
            tc.tile_pool(name="const", bufs=1) as constp,
            tc.tile_pool(name="qt", bufs=NG) as qtp,
            tc.tile_pool(name="qf", bufs=6) as qfp,
            tc.tile_pool(name="mt", bufs=2) as mtp,
            tc.tile_pool(name="outp", bufs=2) as outp,
            tc.tile_pool(name="pall", bufs=1,
                         space=bass.MemorySpace.PSUM) as pallp,
            tc.tile_pool(name="out2", bufs=2,
                         space=bass.MemorySpace.PSUM) as out2p,
        ):
            # x_diag [128, KT*128]: zeros except [32m+pp, 132*kt + m] = x.
            # Build as memset + 4 strided lattice DMAs: only the 4096
            # nonzeros cross the DMA queue, not the 1 MB zero-filled image.
            xd_t = constp.tile([P, KT * P], fp16, name="xd_t")
            nc.vector.memset(xd_t, 0.0)
            for m in range(4):
                nc.sync.dma_start(
                    out=xd_t[32 * m:32 * m + 32, m::132],
                    in_=xds_d[32 * m:32 * m + 32, m::132])

            if passes > 1:
                acc = outp.tile([NCH, CH], f32, name="acc")
                nc.vector.memset(acc, 0.0)

            for rep in range(passes):
                pall = pallp.tile([P, NCH, 512], f32, name="pall")
                # issue all q DMAs up front so transfers queue back-to-back
                qtiles = []
                for g in range(NG):
                    qtile = qtp.tile([P, G, ROWS], i8, name="qtile")
                    nc.sync.dma_start(out=qtile, in_=qt_d[g])
                    qtiles.append(qtile)
                if rep == 0:
                    # stage-2 constants ride behind the first q groups
                    sct_t = constp.tile([P, ROWS], fp16, name="sct_t")
                    nc.sync.dma_start(out=sct_t, in_=sct_d)
                    ones_t = constp.tile([P, NCH * 4], fp16, name="ones_t")
                    nc.sync.dma_start(out=ones_t, in_=ones_d)
                    bias_t = constp.tile([NCH, CH], f32, name="bias_t")
                    nc.sync.dma_start(out=bias_t, in_=bias_d)

                for g in range(NG):
                    qf = qfp.tile([P, G, ROWS], fp16, name="qf")
                    if g in DVE_GROUPS:
                        nc.vector.tensor_copy(qf, qtiles[g])
                    else:
                        nc.scalar.activation(qf, qtiles[g], Copy)
                    for t in range(G):
                        kt = g * G + t
                        for c in range(NCH):
                            nc.tensor.matmul(
                                out=pall[:, c, 0:CH],
                                lhsT=xd_t[:, kt * P:(kt + 1) * P],
                                rhs=qf[:, t, c * CH:(c + 1) * CH],
                                start=(kt == 0), stop=(kt == KT - 1))

                out2 = out2p.tile([NCH, 512], f32, name="out2")
                for c in range(NCH):
                    mt = mtp.tile([P, CH], fp16, name="mt")
                    nc.vector.tensor_mul(
                        mt, pall[:, c, 0:CH],
                        sct_t[:, c * CH:(c + 1) * CH])
                    nc.tensor.matmul(
                        out=out2[:, 0:CH],
                        lhsT=ones_t[:, c * 4:(c + 1) * 4],
                        rhs=mt,
                        start=(c == 0), stop=(c == NCH - 1))

                ot = outp.tile([NCH, CH], f32, name="ot")
                nc.vector.tensor_add(ot, out2[:, 0:CH], bias_t)
                if passes > 1:
                    nc.vector.tensor_add(acc, acc, ot)

            nc.sync.dma_start(out=out_d, in_=acc if passes > 1 else ot)

    _legalize_sync_waits(nc)
    _NC_CACHE[passes] = nc
    return nc


def _make_in_maps(x, q, scales, bias):
    x = np.asarray(x, dtype=np.float32).reshape(IN_F)
    q = np.asarray(q, dtype=np.int32).reshape(OUT_F, IN_F)
    scales = np.asarray(scales, dtype=np.float32).reshape(OUT_F, NB)
    bias = np.asarray(bias, dtype=np.float32).reshape(OUT_F)

    # x_diag: xds[p, kt, j] = x[kt*128 + p] if j == 4*kt + p//32 else 0
    xf = x.astype(np.float16).reshape(KT, P)  # [kt, p]
    xds = np.zeros((P, KT, P), dtype=np.float16)
    for p in range(P):
        for kt in range(KT):
            xds[p, kt, 4 * kt + p // 32] = xf[kt, p]
    xds = xds.reshape(P, KT * P)

    # stage-2 reduction weights: onesb[p, c*4 + m] = 1.0 if m == c else 0
    onesb = np.zeros((P, NCH, 4), dtype=np.float16)
    for c in range(NCH):
        onesb[:, c, c] = 1.0
    onesb = onesb.reshape(P, NCH * 4)

    in_maps = []
    for core in range(N_CORES):
        r0 = core * ROWS
        qc = q[r0:r0 + ROWS]  # [1792, 4096]
        in_maps.append({
            "qt": np.ascontiguousarray(
                qc.T.astype(np.int8).reshape(NG, G, P, ROWS)
                .transpose(0, 2, 1, 3)),
            "xds": xds,
            "sct": np.ascontiguousarray(
                scales[r0:r0 + ROWS].T.astype(np.float16)),
            "onesb": onesb,
            "bias4": np.ascontiguousarray(
                bias[r0:r0 + ROWS].reshape(NCH, CH)),
        })
    return in_maps


def _gather(results):
    parts = []
    for core in range(N_CORES):
        o = np.asarray(results[core]["out"], dtype=np.float32)  # [NCH, CH]
        parts.append(o.reshape(ROWS))
    return np.concatenate(parts).reshape(1, OUT_F).astype(np.float32)


def kernel(x, q, scales, bias):
    from concourse.bass_utils import run_bass_kernel_spmd

    nc = _build_nc()
    in_maps = _make_in_maps(x, q, scales, bias)
    res = run_bass_kernel_spmd(nc, in_maps, list(range(N_CORES)))
    return _gather(res.results)
